# revision 11
# baseline (speedup 1.0000x reference)
"""GQA attention (RoPE + ALiBi + causal) on 8 trn2 NeuronCores.

Sharding: core c -> batch b = c//4, kv-group g = c%4 (4 q-heads + 1 kv-head
per core, column-sharded Wq/Wk/Wv, row-sharded Wo; host sums the 4 partial
Wo outputs per batch).

v2: bf16 operands everywhere on SBUF (PSUM/scores/biases stay f32), one-pass
projections, per-c interleaved weight+x streaming at startup, rotate-half via
a PE permutation matmul (no SBUF-SBUF DMA), Wo of block bk-1 issued after
projections of bk so the PE stays busy during RoPE, software-pipelined
attention with 3-deep score lookahead, column-restricted diagonal tiles
(fully-masked key columns skipped), triangle-only mask adds, packed per-block
softmax denominators with reciprocal_approx_fast.
"""
import sys

if '/opt/trn_rl_repo' not in sys.path:
    sys.path.insert(0, '/opt/trn_rl_repo')

import numpy as np
import ml_dtypes

BF = ml_dtypes.bfloat16

B, T, D = 2, 2048, 2048
H, KV = 16, 4
HD = D // H          # 128
NREP = H // KV       # 4
KVD = 512            # per-core q width (4 heads x 128)
P = 128
TB = 512             # t-block
NBLK = T // TB       # 4
NC = D // P          # 16 contraction tiles
NJ = T // P          # 16 key tiles
ALIBI_W = 0.1
SCALE = (1.0 - ALIBI_W) / np.sqrt(np.float32(HD))

_cache = {}


def _build():
    from concourse import bacc, mybir
    from concourse.tile import TileContext

    F32 = mybir.dt.float32
    BF16 = mybir.dt.bfloat16
    EXP = mybir.ActivationFunctionType.Exp

    nc = bacc.Bacc()
    xT = nc.declare_dram_parameter("xT", [D, T], BF16, isOutput=False)
    wq = nc.declare_dram_parameter("wq", [D, KVD], BF16, isOutput=False)
    wk = nc.declare_dram_parameter("wk", [D, P], BF16, isOutput=False)
    wv = nc.declare_dram_parameter("wv", [D, P], BF16, isOutput=False)
    wo = nc.declare_dram_parameter("wo", [KVD, D], BF16, isOutput=False)
    cosq = nc.declare_dram_parameter("cosq", [P, T], BF16, isOutput=False)
    sinq = nc.declare_dram_parameter("sinq", [P, T], BF16, isOutput=False)
    cosk = nc.declare_dram_parameter("cosk", [P, T], BF16, isOutput=False)
    sink = nc.declare_dram_parameter("sink", [P, T], BF16, isOutput=False)
    cb = nc.declare_dram_parameter("cb", [P, NREP * NBLK * NJ], F32, isOutput=False)
    maskT = nc.declare_dram_parameter("maskT", [P, P], F32, isOutput=False)
    onesc = nc.declare_dram_parameter("onesc", [P, 1], BF16, isOutput=False)
    idin = nc.declare_dram_parameter("idin", [P, P], BF16, isOutput=False)
    permi = nc.declare_dram_parameter("permi", [P, P], BF16, isOutput=False)
    out = nc.declare_dram_parameter("out", [T, D], BF16, isOutput=True)

    with TileContext(nc) as tc:
        with (
            tc.tile_pool(name="const", bufs=1) as cpool,
            tc.tile_pool(name="kv", bufs=1) as kvpool,
            tc.tile_pool(name="xin", bufs=4) as xpool,
            tc.tile_pool(name="rp", bufs=3) as rpool,
            tc.tile_pool(name="qt", bufs=6) as qpool,
            tc.tile_pool(name="pt", bufs=5) as ptpool,
            tc.tile_pool(name="ot", bufs=5) as opool,
            tc.tile_pool(name="ysb", bufs=3) as ypool,
            tc.tile_pool(name="small", bufs=2) as spool,
            tc.tile_pool(name="ps", bufs=1, space="PSUM") as pss,
        ):
            # ---- small resident constants (tiny DMAs, scalar queue) ----
            cb_sb = cpool.tile([P, NREP * NBLK * NJ], F32)
            nc.scalar.dma_start(out=cb_sb, in_=cb[:, :])
            maskT_sb = cpool.tile([P, P], F32)
            nc.scalar.dma_start(out=maskT_sb, in_=maskT[:, :])
            onesc_sb = cpool.tile([P, 1], BF16)
            nc.scalar.dma_start(out=onesc_sb, in_=onesc[:, :])
            id_sb = cpool.tile([P, P], BF16)
            nc.scalar.dma_start(out=id_sb, in_=idin[:, :])
            perm_sb = cpool.tile([P, P], BF16)
            nc.scalar.dma_start(out=perm_sb, in_=permi[:, :])

            # ---- weight tiles: declared here, streamed per-c inside bk=0 ----
            wq_sb = cpool.tile([P, NC, KVD], BF16)
            wq_r = wq.rearrange("(c p) n -> p c n", p=P)
            wk_sb = cpool.tile([P, NC, P], BF16)
            wk_r = wk.rearrange("(c p) n -> p c n", p=P)
            wv_sb = cpool.tile([P, NC, P], BF16)
            wv_r = wv.rearrange("(c p) n -> p c n", p=P)
            wo_sb = cpool.tile([P, NREP, D], BF16)
            wo_r = wo.rearrange("(h p) e -> p h e", p=P)

            # rope tables (full T), loaded after block-0 weights
            cq_sb = cpool.tile([P, T], BF16)
            sq_sb = cpool.tile([P, T], BF16)
            ck_sb = cpool.tile([P, T], BF16)
            sk_sb = cpool.tile([P, T], BF16)

            kT_sb = kvpool.tile([P, T], BF16)        # roped K, [d, s]
            v_sb = kvpool.tile([P, NJ, P], BF16)     # V tiles, [s, j, d']

            ohs_prev = None
            t0_prev = 0

            def rope(dst, src_ps, cos_sl, sin_sl, nm):
                raw = rpool.tile([P, TB], BF16, tag="raw", name=f"raw{nm}")
                nc.any.tensor_copy(raw, src_ps)
                sw_ps = pss.tile([P, TB], F32, tag="big", bufs=6, name=f"sw{nm}")
                nc.tensor.matmul(sw_ps, perm_sb, raw, start=True, stop=True)
                m1 = rpool.tile([P, TB], BF16, tag="m1", name=f"m1{nm}")
                nc.vector.tensor_mul(m1, raw, cos_sl)
                m2 = rpool.tile([P, TB], BF16, tag="m2", name=f"m2{nm}")
                nc.vector.tensor_mul(m2, sw_ps, sin_sl)
                nc.vector.tensor_add(dst, m1, m2)

            for bk in range(NBLK):
                t0 = bk * TB
                # ---- projections: one pass, 6 PSUM banks ----
                q_ps = [pss.tile([P, TB], F32, tag="big", bufs=6, name=f"qps{bk}_{h}")
                        for h in range(NREP)]
                k_ps = pss.tile([P, TB], F32, tag="big", bufs=6, name=f"kps{bk}")
                v_ps = pss.tile([P, TB], F32, tag="big", bufs=6, name=f"vps{bk}")
                if bk == 0:
                    # batched weight streams: k/v first (needed at c=0, small),
                    # then wq in 4-c chunks so c=0..3 can start early
                    nc.scalar.dma_start(out=wk_sb[:, :, :], in_=wk_r[:, :, :])
                    nc.scalar.dma_start(out=wv_sb[:, :, :], in_=wv_r[:, :, :])
                    for cc in range(0, NC, 4):
                        nc.scalar.dma_start(out=wq_sb[:, cc:cc + 4],
                                            in_=wq_r[:, cc:cc + 4])
                for c in range(NC):
                    xt = xpool.tile([P, TB], BF16, tag="xt", name=f"xt{bk}_{c}")
                    nc.sync.dma_start(out=xt, in_=xT[c * P:(c + 1) * P, t0:t0 + TB])
                    st, sp = (c == 0), (c == NC - 1)
                    nc.tensor.matmul(k_ps, wk_sb[:, c, :], xt, start=st, stop=sp)
                    nc.tensor.matmul(v_ps, wv_sb[:, c, :], xt, start=st, stop=sp)
                    for h in range(NREP):
                        nc.tensor.matmul(q_ps[h], wq_sb[:, c, h * P:(h + 1) * P], xt,
                                         start=st, stop=sp)
                if bk == 0:
                    # background loads: rope tables then wo (needed later)
                    nc.scalar.dma_start(out=ck_sb, in_=cosk[:, :])
                    nc.scalar.dma_start(out=sk_sb, in_=sink[:, :])
                    nc.scalar.dma_start(out=cq_sb, in_=cosq[:, :])
                    nc.scalar.dma_start(out=sq_sb, in_=sinq[:, :])
                    nc.scalar.dma_start(out=wo_sb[:, :, :], in_=wo_r[:, :, :])

                # ---- rope k + q0 first so attention can start early ----
                rope(kT_sb[:, t0:t0 + TB], k_ps, ck_sb[:, t0:t0 + TB],
                     sk_sb[:, t0:t0 + TB], f"k{bk}")
                q_sb = [None] * NREP
                q_sb[0] = qpool.tile([P, TB], BF16, tag="qT", name=f"qT{bk}_0")
                rope(q_sb[0], q_ps[0], cq_sb[:, t0:t0 + TB], sq_sb[:, t0:t0 + TB],
                     f"q{bk}_0")

                # ---- V: copy + transpose to [s, d'] ----
                vtmp = rpool.tile([P, TB], BF16, tag="vtmp", name=f"vtmp{bk}")
                nc.any.tensor_copy(vtmp, v_ps)
                for sj in range(4):
                    vt_ps = pss.tile([P, P], BF16, tag="big", bufs=6, name=f"vtps{bk}_{sj}")
                    nc.tensor.transpose(vt_ps, vtmp[:, sj * P:(sj + 1) * P], id_sb)
                    nc.vector.tensor_copy(v_sb[:, 4 * bk + sj, :], vt_ps)

                for h in range(1, NREP):
                    q_sb[h] = qpool.tile([P, TB], BF16, tag="qT", name=f"qT{bk}_{h}")
                    rope(q_sb[h], q_ps[h], cq_sb[:, t0:t0 + TB], sq_sb[:, t0:t0 + TB],
                         f"q{bk}_{h}")

                # ---- Wo of previous block (fills PE while rope runs) ----
                if ohs_prev is not None:
                    for ts_ in range(4):
                        for e in range(4):
                            y_ps = pss.tile([P, TB], F32, tag="big", bufs=6,
                                            name=f"yps{bk}_{ts_}_{e}")
                            for h in range(NREP):
                                nc.tensor.matmul(
                                    y_ps, ohs_prev[h][:, ts_ * P:(ts_ + 1) * P],
                                    wo_sb[:, h, e * TB:(e + 1) * TB],
                                    start=(h == 0), stop=(h == NREP - 1))
                            y_sb = ypool.tile([P, TB], BF16, tag="ysb",
                                              name=f"y{bk}_{ts_}_{e}")
                            nc.vector.tensor_copy(y_sb, y_ps)
                            nc.gpsimd.dma_start(
                                out=out[t0_prev + ts_ * P:t0_prev + (ts_ + 1) * P,
                                        e * TB:(e + 1) * TB],
                                in_=y_sb)

                # ---- attention: software-pipelined over flat (h, j) ----
                nj = 4 * bk + 4
                flat = [(h, j) for h in range(NREP) for j in range(nj)]
                cs_ps = [pss.tile([1, TB], F32, tag="cs", bufs=2, name=f"cs{bk}_{h}")
                         for h in range(NREP)]
                ot_ps = [pss.tile([P, TB], F32, tag="big", bufs=6, name=f"otps{bk}_{h}")
                         for h in range(NREP)]
                ohs = [None] * NREP

                def issue_s(idx):
                    h, j = flat[idx]
                    dlt = j - 4 * bk
                    c0 = P * dlt if dlt > 0 else 0
                    s_ps = pss.tile([P, TB], F32, tag="big", bufs=6,
                                    name=f"sps{bk}_{h}_{j}")
                    nc.tensor.matmul(s_ps[:, c0:], kT_sb[:, j * P:(j + 1) * P],
                                     q_sb[h][:, c0:], start=True, stop=True)
                    if dlt >= 0:
                        nc.vector.tensor_add(s_ps[:, c0:c0 + P], s_ps[:, c0:c0 + P],
                                             maskT_sb)
                    pt = ptpool.tile([P, TB], BF16, tag="pt", name=f"pt{bk}_{h}_{j}")
                    col = (h * NBLK + bk) * NJ + j
                    nc.scalar.activation(pt[:, c0:], s_ps[:, c0:], EXP,
                                         bias=cb_sb[:, col:col + 1])
                    return pt, c0

                def issue_po(idx, pt, c0):
                    h, j = flat[idx]
                    nc.tensor.matmul(cs_ps[h][:, c0:], onesc_sb, pt[:, c0:],
                                     start=(j == 0), stop=(j == nj - 1),
                                     skip_group_check=True)
                    nc.tensor.matmul(ot_ps[h][:, c0:], v_sb[:, j, :], pt[:, c0:],
                                     start=(j == 0), stop=(j == nj - 1),
                                     skip_group_check=True)
                    if j == nj - 1:
                        rec = spool.tile([1, TB], F32, tag="rec", name=f"rec{bk}_{h}")
                        nc.vector.reciprocal_approx_fast(out=rec, in_=cs_ps[h])
                        rbc = spool.tile([P, TB], F32, tag="rbc", name=f"rbc{bk}_{h}")
                        nc.gpsimd.partition_broadcast(rbc, rec)
                        oh = opool.tile([P, TB], BF16, tag="oh", name=f"oh{bk}_{h}")
                        nc.vector.tensor_mul(oh, ot_ps[h], rbc)
                        ohs[h] = oh

                LOOK = 4
                pend = []
                for idx in range(min(LOOK, len(flat))):
                    pend.append(issue_s(idx))
                for idx in range(len(flat)):
                    if idx + LOOK < len(flat):
                        pend.append(issue_s(idx + LOOK))
                    pt, c0 = pend.pop(0)
                    issue_po(idx, pt, c0)

                ohs_prev = ohs
                t0_prev = t0

            # ---- final block's Wo ----
            for ts_ in range(4):
                for e in range(4):
                    y_ps = pss.tile([P, TB], F32, tag="big", bufs=6,
                                    name=f"ypsF_{ts_}_{e}")
                    for h in range(NREP):
                        nc.tensor.matmul(
                            y_ps, ohs_prev[h][:, ts_ * P:(ts_ + 1) * P],
                            wo_sb[:, h, e * TB:(e + 1) * TB],
                            start=(h == 0), stop=(h == NREP - 1))
                    y_sb = ypool.tile([P, TB], BF16, tag="ysb", name=f"yF_{ts_}_{e}")
                    nc.vector.tensor_copy(y_sb, y_ps)
                    nc.gpsimd.dma_start(
                        out=out[t0_prev + ts_ * P:t0_prev + (ts_ + 1) * P,
                                e * TB:(e + 1) * TB],
                        in_=y_sb)

    nc.compile()
    return nc


def _prep_inputs(x, mask, freqs_cis, alibi_bias, Wq, Wk, Wv, Wo):
    """Host-side prep: transposes, RoPE tables, ALiBi bias decomposition."""
    f64 = np.float64
    idx = np.arange(HD)
    cos_full = freqs_cis[:, idx // 2]                     # [T, 128]
    sin_full = freqs_cis[:, (HD // 2) + idx // 2]         # [T, 128]
    sign = np.where(idx < HD // 2, -1.0, 1.0).astype(np.float32)
    cosT = np.ascontiguousarray(cos_full.T)               # [128, T]
    sinT_signed = np.ascontiguousarray((sin_full * sign[None, :]).T)

    cosq = (cosT * np.float32(SCALE)).astype(BF)
    sinq = (sinT_signed * np.float32(SCALE)).astype(BF)
    cosk = cosT.astype(BF)
    sink = sinT_signed.astype(BF)

    # triangle mask block: key p > query c -> -1e9 (transposed layout)
    pp = np.arange(P)
    maskT = np.where(pp[:, None] > pp[None, :], -1e9, 0.0).astype(np.float32)

    onesc = np.ones((P, 1), BF)
    idin = np.eye(P, dtype=np.float32).astype(BF)
    permi = np.zeros((P, P), np.float32)
    permi[(np.arange(P) + P // 2) % P, np.arange(P)] = 1.0
    permi = permi.astype(BF)

    in_maps = []
    for c in range(8):
        b, g = c // 4, c % 4
        slopes = np.array([-f64(alibi_bias[0, g * NREP + hl, 1, 0]) for hl in range(NREP)])
        pvec = np.arange(P, dtype=f64)
        jvec = np.arange(NJ, dtype=f64)
        # cb[p, h, bk, j] = ALIBI_W*slope*(j*128 + p) - ALIBI_W*slope*(bk*512 + 511)
        bkvec = np.arange(NBLK, dtype=f64)
        cbv = (ALIBI_W * slopes[:, None, None, None]
               * (jvec[None, None, :, None] * P + pvec[None, None, None, :]
                  - (bkvec[None, :, None, None] * TB + (TB - 1))))
        cbm = np.ascontiguousarray(cbv.transpose(3, 0, 1, 2).reshape(P, NREP * NBLK * NJ)).astype(np.float32)
        in_maps.append({
            "xT": np.ascontiguousarray(x[b].T).astype(BF),
            "wq": np.ascontiguousarray(Wq[:, g * KVD:(g + 1) * KVD]).astype(BF),
            "wk": np.ascontiguousarray(Wk[:, g * P:(g + 1) * P]).astype(BF),
            "wv": np.ascontiguousarray(Wv[:, g * P:(g + 1) * P]).astype(BF),
            "wo": np.ascontiguousarray(Wo[g * KVD:(g + 1) * KVD, :]).astype(BF),
            "cosq": cosq, "sinq": sinq, "cosk": cosk, "sink": sink,
            "cb": cbm, "maskT": maskT,
            "onesc": onesc, "idin": idin, "permi": permi,
        })
    return in_maps


def kernel(x, mask, freqs_cis, alibi_bias, Wq, Wk, Wv, Wo, _trace=False, _trace_kwargs=None):
    from concourse.bass_utils import run_bass_kernel_spmd

    if "nc" not in _cache:
        _cache["nc"] = _build()
    nc = _cache["nc"]

    in_maps = _prep_inputs(np.asarray(x, np.float32), np.asarray(mask, np.float32),
                           np.asarray(freqs_cis, np.float32), np.asarray(alibi_bias, np.float32),
                           np.asarray(Wq, np.float32), np.asarray(Wk, np.float32),
                           np.asarray(Wv, np.float32), np.asarray(Wo, np.float32))
    kw = {}
    if _trace:
        kw = dict(trace=True, **(_trace_kwargs or {}))
    res = run_bass_kernel_spmd(nc, in_maps, list(range(8)), **kw)

    full = np.zeros((B, T, D), np.float32)
    for c in range(8):
        full[c // 4] += np.asarray(res.results[c]["out"], np.float32)
    if _trace:
        _cache["last_trace"] = res
    return full


# revision 12
# speedup vs baseline: 1.0014x; 1.0014x over previous
"""GQA attention (RoPE + ALiBi + causal) on 8 trn2 NeuronCores.

Sharding: core c -> batch b = c//4, kv-group g = c%4 (4 q-heads + 1 kv-head
per core, column-sharded Wq/Wk/Wv, row-sharded Wo; host sums the 4 partial
Wo outputs per batch).

v2: bf16 operands everywhere on SBUF (PSUM/scores/biases stay f32), one-pass
projections, per-c interleaved weight+x streaming at startup, rotate-half via
a PE permutation matmul (no SBUF-SBUF DMA), Wo of block bk-1 issued after
projections of bk so the PE stays busy during RoPE, software-pipelined
attention with 3-deep score lookahead, column-restricted diagonal tiles
(fully-masked key columns skipped), triangle-only mask adds, packed per-block
softmax denominators with reciprocal_approx_fast.
"""
import sys

if '/opt/trn_rl_repo' not in sys.path:
    sys.path.insert(0, '/opt/trn_rl_repo')

import numpy as np
import ml_dtypes

BF = ml_dtypes.bfloat16

B, T, D = 2, 2048, 2048
H, KV = 16, 4
HD = D // H          # 128
NREP = H // KV       # 4
KVD = 512            # per-core q width (4 heads x 128)
P = 128
TB = 512             # t-block
NBLK = T // TB       # 4
NC = D // P          # 16 contraction tiles
NJ = T // P          # 16 key tiles
ALIBI_W = 0.1
SCALE = (1.0 - ALIBI_W) / np.sqrt(np.float32(HD))

_cache = {}


def _build():
    from concourse import bacc, mybir
    from concourse.tile import TileContext

    F32 = mybir.dt.float32
    BF16 = mybir.dt.bfloat16
    EXP = mybir.ActivationFunctionType.Exp

    nc = bacc.Bacc()
    xT = nc.declare_dram_parameter("xT", [D, T], BF16, isOutput=False)
    wq = nc.declare_dram_parameter("wq", [P, NC * KVD], BF16, isOutput=False)
    wk = nc.declare_dram_parameter("wk", [P, NC * P], BF16, isOutput=False)
    wv = nc.declare_dram_parameter("wv", [P, NC * P], BF16, isOutput=False)
    wo = nc.declare_dram_parameter("wo", [P, NREP * D], BF16, isOutput=False)
    cosq = nc.declare_dram_parameter("cosq", [P, T], BF16, isOutput=False)
    sinq = nc.declare_dram_parameter("sinq", [P, T], BF16, isOutput=False)
    cosk = nc.declare_dram_parameter("cosk", [P, T], BF16, isOutput=False)
    sink = nc.declare_dram_parameter("sink", [P, T], BF16, isOutput=False)
    cb = nc.declare_dram_parameter("cb", [P, NREP * NBLK * NJ], F32, isOutput=False)
    maskT = nc.declare_dram_parameter("maskT", [P, P], F32, isOutput=False)
    onesc = nc.declare_dram_parameter("onesc", [P, 1], BF16, isOutput=False)
    idin = nc.declare_dram_parameter("idin", [P, P], BF16, isOutput=False)
    permi = nc.declare_dram_parameter("permi", [P, P], BF16, isOutput=False)
    out = nc.declare_dram_parameter("out", [T, D], BF16, isOutput=True)

    with TileContext(nc) as tc:
        with (
            tc.tile_pool(name="const", bufs=1) as cpool,
            tc.tile_pool(name="kv", bufs=1) as kvpool,
            tc.tile_pool(name="xin", bufs=4) as xpool,
            tc.tile_pool(name="rp", bufs=3) as rpool,
            tc.tile_pool(name="qt", bufs=6) as qpool,
            tc.tile_pool(name="pt", bufs=5) as ptpool,
            tc.tile_pool(name="ot", bufs=5) as opool,
            tc.tile_pool(name="ysb", bufs=3) as ypool,
            tc.tile_pool(name="small", bufs=2) as spool,
            tc.tile_pool(name="ps", bufs=1, space="PSUM") as pss,
        ):
            # ---- small resident constants (tiny DMAs, scalar queue) ----
            cb_sb = cpool.tile([P, NREP * NBLK * NJ], F32)
            nc.scalar.dma_start(out=cb_sb, in_=cb[:, :])
            maskT_sb = cpool.tile([P, P], F32)
            nc.scalar.dma_start(out=maskT_sb, in_=maskT[:, :])
            onesc_sb = cpool.tile([P, 1], BF16)
            nc.scalar.dma_start(out=onesc_sb, in_=onesc[:, :])
            id_sb = cpool.tile([P, P], BF16)
            nc.scalar.dma_start(out=id_sb, in_=idin[:, :])
            perm_sb = cpool.tile([P, P], BF16)
            nc.scalar.dma_start(out=perm_sb, in_=permi[:, :])

            # ---- weight tiles: declared here, streamed per-c inside bk=0 ----
            wq_sb = cpool.tile([P, NC, KVD], BF16)
            wq_r = wq.rearrange("p (c n) -> p c n", n=KVD)
            wk_sb = cpool.tile([P, NC, P], BF16)
            wk_r = wk.rearrange("p (c n) -> p c n", n=P)
            wv_sb = cpool.tile([P, NC, P], BF16)
            wv_r = wv.rearrange("p (c n) -> p c n", n=P)
            wo_sb = cpool.tile([P, NREP, D], BF16)
            wo_r = wo.rearrange("p (h e) -> p h e", e=D)

            # rope tables (full T), loaded after block-0 weights
            cq_sb = cpool.tile([P, T], BF16)
            sq_sb = cpool.tile([P, T], BF16)
            ck_sb = cpool.tile([P, T], BF16)
            sk_sb = cpool.tile([P, T], BF16)

            kT_sb = kvpool.tile([P, T], BF16)        # roped K, [d, s]
            v_sb = kvpool.tile([P, NJ, P], BF16)     # V tiles, [s, j, d']

            ohs_prev = None
            t0_prev = 0

            def rope(dst, src_ps, cos_sl, sin_sl, nm):
                raw = rpool.tile([P, TB], BF16, tag="raw", name=f"raw{nm}")
                nc.any.tensor_copy(raw, src_ps)
                sw_ps = pss.tile([P, TB], F32, tag="big", bufs=6, name=f"sw{nm}")
                nc.tensor.matmul(sw_ps, perm_sb, raw, start=True, stop=True)
                m1 = rpool.tile([P, TB], BF16, tag="m1", name=f"m1{nm}")
                nc.vector.tensor_mul(m1, raw, cos_sl)
                m2 = rpool.tile([P, TB], BF16, tag="m2", name=f"m2{nm}")
                nc.vector.tensor_mul(m2, sw_ps, sin_sl)
                nc.vector.tensor_add(dst, m1, m2)

            for bk in range(NBLK):
                t0 = bk * TB
                # ---- projections: one pass, 6 PSUM banks ----
                q_ps = [pss.tile([P, TB], F32, tag="big", bufs=6, name=f"qps{bk}_{h}")
                        for h in range(NREP)]
                k_ps = pss.tile([P, TB], F32, tag="big", bufs=6, name=f"kps{bk}")
                v_ps = pss.tile([P, TB], F32, tag="big", bufs=6, name=f"vps{bk}")
                if bk == 0:
                    # chunked weight streams so c=0 can start early
                    nc.scalar.dma_start(out=wk_sb[:, 0:8], in_=wk_r[:, 0:8])
                    nc.scalar.dma_start(out=wv_sb[:, 0:8], in_=wv_r[:, 0:8])
                    nc.scalar.dma_start(out=wq_sb[:, 0:4], in_=wq_r[:, 0:4])
                    nc.scalar.dma_start(out=wk_sb[:, 8:16], in_=wk_r[:, 8:16])
                    nc.scalar.dma_start(out=wv_sb[:, 8:16], in_=wv_r[:, 8:16])
                    for cc in range(4, NC, 4):
                        nc.scalar.dma_start(out=wq_sb[:, cc:cc + 4],
                                            in_=wq_r[:, cc:cc + 4])
                for c in range(NC):
                    xt = xpool.tile([P, TB], BF16, tag="xt", name=f"xt{bk}_{c}")
                    nc.sync.dma_start(out=xt, in_=xT[c * P:(c + 1) * P, t0:t0 + TB])
                    st, sp = (c == 0), (c == NC - 1)
                    nc.tensor.matmul(k_ps, wk_sb[:, c, :], xt, start=st, stop=sp)
                    nc.tensor.matmul(v_ps, wv_sb[:, c, :], xt, start=st, stop=sp)
                    for h in range(NREP):
                        nc.tensor.matmul(q_ps[h], wq_sb[:, c, h * P:(h + 1) * P], xt,
                                         start=st, stop=sp)
                if bk == 0:
                    # background loads: rope tables then wo (needed later)
                    nc.scalar.dma_start(out=ck_sb, in_=cosk[:, :])
                    nc.scalar.dma_start(out=sk_sb, in_=sink[:, :])
                    nc.scalar.dma_start(out=cq_sb, in_=cosq[:, :])
                    nc.scalar.dma_start(out=sq_sb, in_=sinq[:, :])
                    nc.scalar.dma_start(out=wo_sb[:, 0:2], in_=wo_r[:, 0:2])
                    nc.scalar.dma_start(out=wo_sb[:, 2:4], in_=wo_r[:, 2:4])

                # ---- rope k + q0 first so attention can start early ----
                rope(kT_sb[:, t0:t0 + TB], k_ps, ck_sb[:, t0:t0 + TB],
                     sk_sb[:, t0:t0 + TB], f"k{bk}")
                q_sb = [None] * NREP
                q_sb[0] = qpool.tile([P, TB], BF16, tag="qT", name=f"qT{bk}_0")
                rope(q_sb[0], q_ps[0], cq_sb[:, t0:t0 + TB], sq_sb[:, t0:t0 + TB],
                     f"q{bk}_0")

                # ---- V: copy + transpose to [s, d'] ----
                vtmp = rpool.tile([P, TB], BF16, tag="vtmp", name=f"vtmp{bk}")
                nc.any.tensor_copy(vtmp, v_ps)
                for sj in range(4):
                    vt_ps = pss.tile([P, P], BF16, tag="big", bufs=6, name=f"vtps{bk}_{sj}")
                    nc.tensor.transpose(vt_ps, vtmp[:, sj * P:(sj + 1) * P], id_sb)
                    nc.vector.tensor_copy(v_sb[:, 4 * bk + sj, :], vt_ps)

                for h in range(1, NREP):
                    q_sb[h] = qpool.tile([P, TB], BF16, tag="qT", name=f"qT{bk}_{h}")
                    rope(q_sb[h], q_ps[h], cq_sb[:, t0:t0 + TB], sq_sb[:, t0:t0 + TB],
                         f"q{bk}_{h}")

                # ---- Wo of previous block (fills PE while rope runs) ----
                if ohs_prev is not None:
                    for ts_ in range(4):
                        for e in range(4):
                            y_ps = pss.tile([P, TB], F32, tag="big", bufs=6,
                                            name=f"yps{bk}_{ts_}_{e}")
                            for h in range(NREP):
                                nc.tensor.matmul(
                                    y_ps, ohs_prev[h][:, ts_ * P:(ts_ + 1) * P],
                                    wo_sb[:, h, e * TB:(e + 1) * TB],
                                    start=(h == 0), stop=(h == NREP - 1))
                            y_sb = ypool.tile([P, TB], BF16, tag="ysb",
                                              name=f"y{bk}_{ts_}_{e}")
                            nc.vector.tensor_copy(y_sb, y_ps)
                            nc.gpsimd.dma_start(
                                out=out[t0_prev + ts_ * P:t0_prev + (ts_ + 1) * P,
                                        e * TB:(e + 1) * TB],
                                in_=y_sb)

                # ---- attention: software-pipelined over flat (h, j) ----
                nj = 4 * bk + 4
                flat = [(h, j) for h in range(NREP) for j in range(nj)]
                cs_ps = [pss.tile([1, TB], F32, tag="cs", bufs=2, name=f"cs{bk}_{h}")
                         for h in range(NREP)]
                ot_ps = [pss.tile([P, TB], F32, tag="big", bufs=6, name=f"otps{bk}_{h}")
                         for h in range(NREP)]
                ohs = [None] * NREP

                def issue_s(idx):
                    h, j = flat[idx]
                    dlt = j - 4 * bk
                    c0 = P * dlt if dlt > 0 else 0
                    s_ps = pss.tile([P, TB], F32, tag="big", bufs=6,
                                    name=f"sps{bk}_{h}_{j}")
                    nc.tensor.matmul(s_ps[:, c0:], kT_sb[:, j * P:(j + 1) * P],
                                     q_sb[h][:, c0:], start=True, stop=True)
                    if dlt >= 0:
                        nc.vector.tensor_add(s_ps[:, c0:c0 + P], s_ps[:, c0:c0 + P],
                                             maskT_sb)
                    pt = ptpool.tile([P, TB], BF16, tag="pt", name=f"pt{bk}_{h}_{j}")
                    col = (h * NBLK + bk) * NJ + j
                    nc.scalar.activation(pt[:, c0:], s_ps[:, c0:], EXP,
                                         bias=cb_sb[:, col:col + 1])
                    return pt, c0

                def issue_po(idx, pt, c0):
                    h, j = flat[idx]
                    nc.tensor.matmul(cs_ps[h][:, c0:], onesc_sb, pt[:, c0:],
                                     start=(j == 0), stop=(j == nj - 1),
                                     skip_group_check=True)
                    nc.tensor.matmul(ot_ps[h][:, c0:], v_sb[:, j, :], pt[:, c0:],
                                     start=(j == 0), stop=(j == nj - 1),
                                     skip_group_check=True)
                    if j == nj - 1:
                        rec = spool.tile([1, TB], F32, tag="rec", name=f"rec{bk}_{h}")
                        nc.vector.reciprocal_approx_fast(out=rec, in_=cs_ps[h])
                        rbc = spool.tile([P, TB], F32, tag="rbc", name=f"rbc{bk}_{h}")
                        nc.gpsimd.partition_broadcast(rbc, rec)
                        oh = opool.tile([P, TB], BF16, tag="oh", name=f"oh{bk}_{h}")
                        nc.vector.tensor_mul(oh, ot_ps[h], rbc)
                        ohs[h] = oh

                LOOK = 3
                pend = []
                for idx in range(min(LOOK, len(flat))):
                    pend.append(issue_s(idx))
                for idx in range(len(flat)):
                    if idx + LOOK < len(flat):
                        pend.append(issue_s(idx + LOOK))
                    pt, c0 = pend.pop(0)
                    issue_po(idx, pt, c0)

                ohs_prev = ohs
                t0_prev = t0

            # ---- final block's Wo ----
            for ts_ in range(4):
                for e in range(4):
                    y_ps = pss.tile([P, TB], F32, tag="big", bufs=6,
                                    name=f"ypsF_{ts_}_{e}")
                    for h in range(NREP):
                        nc.tensor.matmul(
                            y_ps, ohs_prev[h][:, ts_ * P:(ts_ + 1) * P],
                            wo_sb[:, h, e * TB:(e + 1) * TB],
                            start=(h == 0), stop=(h == NREP - 1))
                    y_sb = ypool.tile([P, TB], BF16, tag="ysb", name=f"yF_{ts_}_{e}")
                    nc.vector.tensor_copy(y_sb, y_ps)
                    nc.gpsimd.dma_start(
                        out=out[t0_prev + ts_ * P:t0_prev + (ts_ + 1) * P,
                                e * TB:(e + 1) * TB],
                        in_=y_sb)

    nc.compile()
    return nc


def _pack_w(w):
    """[C*128, N] -> [128, C*N] with row p holding w[c*128+p, :] for each c."""
    cN, n = w.shape
    c = cN // P
    return np.ascontiguousarray(
        w.reshape(c, P, n).transpose(1, 0, 2).reshape(P, c * n)).astype(BF)


def _prep_inputs(x, mask, freqs_cis, alibi_bias, Wq, Wk, Wv, Wo):
    """Host-side prep: transposes, RoPE tables, ALiBi bias decomposition."""
    f64 = np.float64
    idx = np.arange(HD)
    cos_full = freqs_cis[:, idx // 2]                     # [T, 128]
    sin_full = freqs_cis[:, (HD // 2) + idx // 2]         # [T, 128]
    sign = np.where(idx < HD // 2, -1.0, 1.0).astype(np.float32)
    cosT = np.ascontiguousarray(cos_full.T)               # [128, T]
    sinT_signed = np.ascontiguousarray((sin_full * sign[None, :]).T)

    cosq = (cosT * np.float32(SCALE)).astype(BF)
    sinq = (sinT_signed * np.float32(SCALE)).astype(BF)
    cosk = cosT.astype(BF)
    sink = sinT_signed.astype(BF)

    # triangle mask block: key p > query c -> -1e9 (transposed layout)
    pp = np.arange(P)
    maskT = np.where(pp[:, None] > pp[None, :], -1e9, 0.0).astype(np.float32)

    onesc = np.ones((P, 1), BF)
    idin = np.eye(P, dtype=np.float32).astype(BF)
    permi = np.zeros((P, P), np.float32)
    permi[(np.arange(P) + P // 2) % P, np.arange(P)] = 1.0
    permi = permi.astype(BF)

    in_maps = []
    for c in range(8):
        b, g = c // 4, c % 4
        slopes = np.array([-f64(alibi_bias[0, g * NREP + hl, 1, 0]) for hl in range(NREP)])
        pvec = np.arange(P, dtype=f64)
        jvec = np.arange(NJ, dtype=f64)
        # cb[p, h, bk, j] = ALIBI_W*slope*(j*128 + p) - ALIBI_W*slope*(bk*512 + 511)
        bkvec = np.arange(NBLK, dtype=f64)
        cbv = (ALIBI_W * slopes[:, None, None, None]
               * (jvec[None, None, :, None] * P + pvec[None, None, None, :]
                  - (bkvec[None, :, None, None] * TB + (TB - 1))))
        cbm = np.ascontiguousarray(cbv.transpose(3, 0, 1, 2).reshape(P, NREP * NBLK * NJ)).astype(np.float32)
        in_maps.append({
            "xT": np.ascontiguousarray(x[b].T).astype(BF),
            "wq": _pack_w(Wq[:, g * KVD:(g + 1) * KVD]),
            "wk": _pack_w(Wk[:, g * P:(g + 1) * P]),
            "wv": _pack_w(Wv[:, g * P:(g + 1) * P]),
            "wo": _pack_w(Wo[g * KVD:(g + 1) * KVD, :]),
            "cosq": cosq, "sinq": sinq, "cosk": cosk, "sink": sink,
            "cb": cbm, "maskT": maskT,
            "onesc": onesc, "idin": idin, "permi": permi,
        })
    return in_maps


def kernel(x, mask, freqs_cis, alibi_bias, Wq, Wk, Wv, Wo, _trace=False, _trace_kwargs=None):
    from concourse.bass_utils import run_bass_kernel_spmd

    if "nc" not in _cache:
        _cache["nc"] = _build()
    nc = _cache["nc"]

    in_maps = _prep_inputs(np.asarray(x, np.float32), np.asarray(mask, np.float32),
                           np.asarray(freqs_cis, np.float32), np.asarray(alibi_bias, np.float32),
                           np.asarray(Wq, np.float32), np.asarray(Wk, np.float32),
                           np.asarray(Wv, np.float32), np.asarray(Wo, np.float32))
    kw = {}
    if _trace:
        kw = dict(trace=True, **(_trace_kwargs or {}))
    res = run_bass_kernel_spmd(nc, in_maps, list(range(8)), **kw)

    full = np.zeros((B, T, D), np.float32)
    for c in range(8):
        full[c // 4] += np.asarray(res.results[c]["out"], np.float32)
    if _trace:
        _cache["last_trace"] = res
    return full
